# revision 42
# baseline (speedup 1.0000x reference)
"""MoE experts kernel for TRN2, expert-parallel over 8 NeuronCores.

Reference computation (T=4096, E=8, H=1024, Q=1024):
    gate_up = einsum('th,ehq->teq', x, gate_up_proj)      # (T, E, 2Q)
    gate, up = split(gate_up, 2, axis=-1)
    hidden = silu(gate) * up                              # (T, E, Q)
    expert_outputs = einsum('teq,eqh->teh', hidden, down_proj)
    out = einsum('teh,te->th', expert_outputs, routing_weights)

Sharding: expert-parallel. Core e computes its expert's full contribution
r[:, e] * (silu(x @ Wgu_gate) * (x @ Wgu_up)) @ Wdn  for all T tokens,
entirely in feature-major layout (features on partitions, tokens on the
free axis) so no on-device transposes are needed; the host sums the 8
partial outputs (the expert-parallel all-reduce) and transposes back.

Per-core cost model (measured):
  - 1536 bf16 matmuls of [128 contraction x 512 moving] at ~216ns each
    = 332us PE busy; 216ns is the effective clock floor (same-stationary
    matmuls are no faster). fp8 is no help twice over: its quantization
    error (3.8-6.5% end-to-end, 1.42% even for a single DR k-tile slice,
    measured in-situ) eats the 2e-2 gate's margin, AND DoubleRow perf
    mode measures >= 1 cycle/row on this hardware path (no double
    pumping). bf16 end-to-end measures 4.4e-3.
  - dma_start descriptor generation occupies the issuing engine ~2.3ns
    per 1KB-row descriptor (~430GB/s/ring); SP and Activation are the two
    HWDGE rings. Weights go on SP, x-stream on Activation, stores split
    across both. The startup HBM window is kept exclusive to the first
    weight slabs + first token chunk; everything else is emitted
    mid-chunk-0 or at chunk bottoms.
  - ~7us fixed framework prologue before any DMA issues; ~1.3us DMA
    completion-to-semaphore latency; ~1.8us epilogue barrier.
"""

import sys

for _p in ("/opt/trn_rl_repo", "/root/.axon_site/_ro/trn_rl_repo"):
    if _p not in sys.path:
        sys.path.insert(0, _p)

import numpy as np

T, E, H, Q = 4096, 8, 1024, 1024
P = 128          # partitions
TC = 512         # token chunk (= one PSUM bank of fp32)
NT = T // TC     # 8 token chunks
KH = H // P      # 8 contraction tiles for the gate_up matmul
KQ = Q // P      # 8 contraction tiles for the down matmul
NH = H // P      # 8 output-feature tiles

_CACHED = None


def _split_waits(nc, max_waits=1):
    """Walrus codegen for several TRN2 ISA structs accepts only one sync-wait
    per instruction ("Too many sync wait commands"). Splitting is safe: a
    same-engine NoOp earlier in the (FIFO) stream carrying the extra waits
    blocks the stream at the same point the original multi-wait would have."""
    import concourse.mybir as mybir

    for f in nc.m.functions:
        for blk in f.blocks:
            newlist, changed = [], False
            for inst in blk.instructions:
                si = inst.sync_info
                if si is not None and si.on_wait and len(si.on_wait) > max_waits:
                    extra = si.on_wait[:-max_waits]
                    keep = si.on_wait[-max_waits:]
                    inst.sync_info = mybir.SyncInfo(
                        on_wait=list(keep), on_update=list(si.on_update or [])
                    )
                    for j, w in enumerate(extra):
                        nop = mybir.InstNoOp(
                            name=f"{inst.name}-wn{j}", engine=inst.engine
                        )
                        nop.sync_info = mybir.SyncInfo(on_wait=[w], on_update=[])
                        newlist.append(nop)
                    changed = True
                newlist.append(inst)
            if changed:
                blk.instructions = newlist


def _dedup_waits(nc):
    """Drop a sync-wait (sem >= v) when an earlier instruction in the same
    engine stream already waited on (sem >= v') with v' >= v. Engine streams
    are FIFO and bass DMA/compute semaphores count monotonically upward
    within a kernel, so the earlier wait already guarantees the condition."""
    import concourse.mybir as mybir

    for f in nc.m.functions:
        for blk in f.blocks:
            seen = {}  # engine -> {sem_id: max_value_waited}
            for inst in blk.instructions:
                si = inst.sync_info
                if si is None or not si.on_wait:
                    continue
                eng_seen = seen.setdefault(inst.engine, {})
                kept = []
                for w in si.on_wait:
                    ge = str(w.wait_mode) == "sem-ge-imm" and w.uses_immediate
                    if ge and eng_seen.get(w.id, -1) >= w.wait_value:
                        continue
                    if ge:
                        eng_seen[w.id] = max(eng_seen.get(w.id, -1), w.wait_value)
                    kept.append(w)
                if len(kept) != len(si.on_wait):
                    inst.sync_info = mybir.SyncInfo(
                        on_wait=kept, on_update=list(si.on_update or [])
                    )


def _build():
    import concourse.bass as bass
    import concourse.mybir as mybir
    import concourse.tile as tile

    nc = bass.Bass("TRN2", target_bir_lowering=False, debug=False, num_devices=E)

    f32 = mybir.dt.float32
    # bf16: same PE rate as fp32r (1 cycle/row for moving >= 256) but half
    # the HBM traffic and half-width weight loads; quantization adds ~0.3%
    # relative error, well inside the 2e-2 gate.
    bf16 = mybir.dt.bfloat16

    # xT viewed as [KH, P, T] so one (p, k, t) DMA per token chunk covers all
    # contraction tiles: descriptor generation on the issuing engine costs
    # ~600ns per dma_start, so few big DMAs beat many small ones.
    xT_d = nc.dram_tensor("xT", [KH, P, T], bf16, kind="ExternalInput").ap()
    # w_gu host-packed as (2Q/P slabs, P, KH, P): slab order is first-use
    # order (gate qi, up qi alternating); each slab is one contiguous DMA
    # with 2KB per partition.
    wgu_d = nc.dram_tensor(
        "w_gu", [2 * Q // P, P, KH, P], bf16, kind="ExternalInput"
    ).ap()
    wdn_d = nc.dram_tensor("w_dn", [KQ, P, H], bf16, kind="ExternalInput").ap()
    rw_d = nc.dram_tensor("rw", [1, T], mybir.dt.float32, kind="ExternalInput").ap()
    # bf16 output: the host upcasts and sums the 8 expert partials in fp32;
    # the extra ~0.2% quantization is inside the 2e-2 budget and halves the
    # store traffic (shrinks the end-of-kernel DMA drain).
    out_d = nc.dram_tensor("out", [NH, P, T], bf16, kind="ExternalOutput").ap()

    from contextlib import ExitStack

    with tile.TileContext(nc) as tc:
        with ExitStack() as es:
            consts = es.enter_context(tc.tile_pool(name="consts", bufs=1))
            psum_gu = es.enter_context(tc.tile_pool(name="psum_gu", bufs=2, space="PSUM"))
            psum_o = es.enter_context(tc.tile_pool(name="psum_o", bufs=4, space="PSUM"))
            xT_pool = es.enter_context(tc.tile_pool(name="xTc", bufs=2))
            hid_pool = es.enter_context(tc.tile_pool(name="hid", bufs=2))
            tmp_pool = es.enter_context(tc.tile_pool(name="tmp", bufs=2))
            ost_pool = es.enter_context(tc.tile_pool(name="ost", bufs=2))
            wgu_s = consts.tile([P, 2 * Q // P, KH, P], bf16)
            wdn_s = consts.tile([P, KQ, H], bf16)

            # PE p-state warmup: the engine idles from the end of the
            # framework prologue (~7us) until the first weights land
            # (~10.5us); matmuls on an uninitialized scratch tile during that
            # window ramp the clock so the real stream starts at full speed.
            # Results land in a PSUM bank that every real accumulation group
            # resets with start=True.
            dmy = consts.tile([P, 4, P], bf16)
            nc.gpsimd.memset(dmy, 0)
            warm_ps = psum_gu.tile([P, TC], f32, tag="gate")
            # 8 dummies x ~427ns (ramp speed) ~= the [8.1us, 11.3us] idle
            # window before the first weights land; more would delay the
            # real stream. (fp8 DoubleRow was probed here; no clear 2x, and
            # the 2e-2 error gate independently rules fp8 out.)
            for _ in range(8):
                nc.tensor.matmul(
                    warm_ps, dmy[:, 0, :], dmy.rearrange("p a b -> p (a b)"),
                    start=True, stop=True,
                )

            # Two independent HWDGE rings on TRN2: SP (nc.sync) carries
            # weights, Activation (nc.scalar) carries the x token-chunk
            # stream; output stores alternate rings. Descriptor generation on
            # the issuing engine runs at only ~430GB/s (~2.3ns/descriptor),
            # so: fine-grained pieces at startup (PE chases individual
            # completions), batched loads once there's slack.
            # Startup window [7us, ~20us] is HBM-critical for the weight
            # slabs: keep it slab-exclusive. Only xc0 shares it (needed
            # immediately); xc1 / routing bcast / prefetches are emitted
            # mid-chunk-0 below, once the slab burst is done.
            nc.sync.dma_start(out=wgu_s[:, 0, 0:KH // 2], in_=wgu_d[0, :, 0:KH // 2])
            nc.sync.dma_start(out=wgu_s[:, 0, KH // 2:], in_=wgu_d[0, :, KH // 2:])
            xT_chunks = []
            xc0 = xT_pool.tile([P, KH, TC], bf16, tag="xc")
            # xc0 in 2-k-tile pieces, all on the ACT ring: 256KB pieces land
            # ahead of the PE's k-sweep (a 512KB half stalls mm#4 by ~1.3us;
            # putting any piece on SP delays the early weight slabs more
            # than it helps — both measured)
            for k0 in range(0, KH, 2):
                nc.scalar.dma_start(
                    out=xc0[:, k0:k0 + 2, :],
                    in_=xT_d[k0:k0 + 2, :, 0:TC].rearrange("k p t -> p k t"),
                )
            xT_chunks.append(xc0)
            for s in (1, 2, 3):
                nc.sync.dma_start(out=wgu_s[:, s], in_=wgu_d[s])
            # slabs 4-7 singly: batched completion semaphores make the PE
            # wait on the whole batch for the first slab it needs (measured
            # ~1.2us stalls at slab 4 and slab 7)
            for s in (4, 5, 6, 7):
                nc.sync.dma_start(out=wgu_s[:, s], in_=wgu_d[s])
            for s0 in range(8, 2 * Q // P, 4):
                nc.sync.dma_start(
                    out=wgu_s[:, s0:s0 + 4],
                    in_=wgu_d[s0:s0 + 4].rearrange("s p k c -> p s k c"),
                )
            nc.sync.dma_start(
                out=wdn_s, in_=wdn_d.rearrange("k p h -> p k h")
            )
            r_all = consts.tile([P, T], f32)
            xc1 = xT_pool.tile([P, KH, TC], bf16, tag="xc")
            xT_chunks.append(xc1)

            for tci in range(NT):
                t0 = tci * TC
                xc = xT_chunks[tci]
                r_c = r_all[:, t0:t0 + TC]
                hid = hid_pool.tile([P, KQ, TC], bf16)
                for qi in range(KQ):
                    if tci == 0 and qi == 4:
                        # slab burst done by now: stream in chunk 1 and the
                        # routing broadcast behind it
                        for k0 in range(0, KH, 4):
                            nc.scalar.dma_start(
                                out=xc1[:, k0:k0 + 4, :],
                                in_=xT_d[k0:k0 + 4, :, TC:2 * TC].rearrange(
                                    "k p t -> p k t"
                                ),
                            )
                        nc.scalar.dma_start(
                            out=r_all, in_=rw_d.to_broadcast([P, T])
                        )
                    gate_ps = psum_gu.tile([P, TC], f32, tag="gate")
                    up_ps = psum_gu.tile([P, TC], f32, tag="up")
                    for k in range(KH):
                        nc.tensor.matmul(
                            gate_ps,
                            wgu_s[:, 2 * qi, k, :],
                            xc[:, k, :],
                            start=(k == 0),
                            stop=(k == KH - 1),
                        )
                    for k in range(KH):
                        nc.tensor.matmul(
                            up_ps,
                            wgu_s[:, 2 * qi + 1, k, :],
                            xc[:, k, :],
                            start=(k == 0),
                            stop=(k == KH - 1),
                        )
                    tmp = tmp_pool.tile([P, TC], f32)
                    nc.scalar.activation(
                        tmp, gate_ps, mybir.ActivationFunctionType.Silu
                    )
                    nc.vector.tensor_mul(hid[:, qi, :], tmp, up_ps)

                ost = ost_pool.tile([P, NH, TC], bf16, tag="ost")
                for hi in range(NH):
                    o_ps = psum_o.tile([P, TC], f32)
                    for qi in range(KQ):
                        nc.tensor.matmul(
                            o_ps,
                            wdn_s[:, qi, hi * P:(hi + 1) * P],
                            hid[:, qi, :],
                            start=(qi == 0),
                            stop=(qi == KQ - 1),
                        )
                    nc.vector.tensor_mul(ost[:, hi, :], o_ps, r_c)
                    # stores: hi 0-3 on SP, 4-6 on ACT, 7 alone on SP — the
                    # two rings drain in parallel and the last piece is small
                    if hi == 3:
                        nc.sync.dma_start(
                            out=out_d[0:4, :, t0:t0 + TC].rearrange("h p t -> p h t"),
                            in_=ost[:, 0:4, :],
                        )
                    elif hi == 6:
                        nc.scalar.dma_start(
                            out=out_d[4:7, :, t0:t0 + TC].rearrange("h p t -> p h t"),
                            in_=ost[:, 4:7, :],
                        )
                    elif hi == 7:
                        nc.sync.dma_start(
                            out=out_d[7:8, :, t0:t0 + TC].rearrange("h p t -> p h t"),
                            in_=ost[:, 7:8, :],
                        )
                # prefetch x two chunks ahead; emitted after the chunk's
                # compute so its descriptor gen never delays startup-critical
                # loads or this chunk's silus
                if 2 <= tci + 2 < NT:
                    nxc = xT_pool.tile([P, KH, TC], bf16, tag="xc")
                    nc.scalar.dma_start(
                        out=nxc,
                        in_=xT_d[:, :, (tci + 2) * TC:(tci + 3) * TC].rearrange(
                            "k p t -> p k t"
                        ),
                    )
                    xT_chunks.append(nxc)
    _split_waits(nc)
    return nc


def _get_nc():
    global _CACHED
    if _CACHED is None:
        _CACHED = _build()
    return _CACHED


def _pack_wgu(w):
    """(H, 2Q) -> (16, 128, KH, 128) bf16 slabs in first-use order:
    128-column blocks interleaved gate qi / up qi, each slab
    partition-major."""
    import ml_dtypes

    w = np.asarray(w, dtype=np.float32)
    # (KH, P, n_blk, P): k-tile, partition, column block, column
    w4 = w.reshape(KH, P, 2 * Q // P, P)
    order = [b for qi in range(KQ) for b in (qi, KQ + qi)]
    # slab s: (P, KH, P)
    return np.ascontiguousarray(
        w4.transpose(2, 1, 0, 3)[order].astype(ml_dtypes.bfloat16)
    )


def _make_in_maps(x, routing_weights, gate_up_proj, down_proj):
    import ml_dtypes

    xT = np.ascontiguousarray(
        np.asarray(x, dtype=np.float32).T.astype(ml_dtypes.bfloat16)
    )
    rw = np.asarray(routing_weights, dtype=np.float32)
    in_maps = []
    for e in range(E):
        in_maps.append({
            "xT": xT,
            "w_gu": _pack_wgu(gate_up_proj[e]),
            "w_dn": np.ascontiguousarray(
                np.asarray(down_proj[e], dtype=np.float32).astype(ml_dtypes.bfloat16)
            ),
            "rw": np.ascontiguousarray(rw[:, e].reshape(1, T)),
        })
    return in_maps


def _reduce_out(res):
    total = np.zeros((H, T), dtype=np.float32)
    for r in res.results:
        total += r["out"].astype(np.float32).reshape(H, T)
    return np.ascontiguousarray(total.T)


def kernel(x, routing_weights, gate_up_proj, down_proj):
    from concourse.bass_utils import run_bass_kernel_spmd

    nc = _get_nc()
    in_maps = _make_in_maps(x, routing_weights, gate_up_proj, down_proj)
    res = run_bass_kernel_spmd(nc, in_maps, core_ids=list(range(E)))
    return _reduce_out(res)

